# revision 6
# baseline (speedup 1.0000x reference)
"""Multi-head attention (B=4, S=2048, D=1024, H=16) on 8 TRN2 NeuronCores.

Sharding: core c handles batch b = c//2 and head-group hg = c%2 (8 heads,
d-slice of width 512). Each core computes QKV projections for its slice,
scores + softmax + probs output for its 8 heads, attention output, and a
partial Wo projection. Host sums the two partial y's per batch and adds the
(bv @ Wo + bo) constant.

On-device pipeline (all matmuls bf16, fp32 PSUM accumulation):
  phase 1: Q^T = (Wq')^T-proj in [d, s] layout (scale 1/8 folded into Wq/bq),
           K^T likewise, V in [s, d] layout.
  phase 2 per (head, q-tile of 128):
           scores psum [128, S] <- 4 matmuls (contraction d_k=64)
           exp on ACT (psum -> sbuf bf16) with accum_out row sums
           reciprocal on DVE, tensor_scalar normalize (bf16, 4x mode)
           probs out via SWDGE cast-DMA (bf16 sbuf -> fp32 DRAM)
           dma_start_transpose(probs) -> P^T tiles for the AV matmul
           AV: out^T[d, q] += V.T-slices @ P^T (col-packed head pairs)
           proj: y[q, e] += avn.T @ Wo-slices, fp32 partial out.
"""

import os
import numpy as np
import ml_dtypes

os.environ.setdefault("MYCRO_LOCAL_CACHE", "1")

N_CORES = 8
B, S_FULL, D_MODEL, NUM_HEADS, D_K = 4, 2048, 1024, 16, 64
BF16 = ml_dtypes.bfloat16


def build_bass(S=2048, DM=1024, HC=8, num_devices=N_CORES):
    """Build the per-core bass program. HC = heads per core."""
    import concourse.mybir as mybir
    import concourse.tile as tile
    from concourse import bacc

    f32 = mybir.dt.float32
    bf16 = mybir.dt.bfloat16
    Exp = mybir.ActivationFunctionType.Exp
    Ident = mybir.ActivationFunctionType.Identity
    mult = mybir.AluOpType.mult

    DH = HC * D_K            # this core's d-slice width (512)
    CT = DM // 128           # contraction tiles for projections (8)
    HP = HC // 2             # head pairs == d-tiles of the slice (4)
    ST = S // 128            # s-tiles (16)
    SC = S // 512            # 512-wide s-chunks (4)
    QT = S // 128            # q-tiles (16)
    EC = DM // 512           # output-proj 512-chunks (2)

    nc = bacc.Bacc("TRN2", target_bir_lowering=False, debug=False,
                   num_devices=num_devices)

    xT = nc.dram_tensor("xT", [DM, S], bf16, kind="ExternalInput").ap()
    wq = nc.dram_tensor("wq", [DM, DH], bf16, kind="ExternalInput").ap()
    wk = nc.dram_tensor("wk", [DM, DH], bf16, kind="ExternalInput").ap()
    wv = nc.dram_tensor("wv", [DM, DH], bf16, kind="ExternalInput").ap()
    wo = nc.dram_tensor("wo", [DH, DM], bf16, kind="ExternalInput").ap()
    bq = nc.dram_tensor("bq", [DH], f32, kind="ExternalInput").ap()
    bk = nc.dram_tensor("bk", [DH], f32, kind="ExternalInput").ap()
    probs_p = nc.dram_tensor("probs_p", [HC, S, S], f32,
                             kind="ExternalOutput").ap()
    y_p = nc.dram_tensor("y_p", [S, DM], f32, kind="ExternalOutput").ap()

    with tile.TileContext(nc) as tc, \
         tc.tile_pool(name="persist", bufs=1) as persist:
        qT_sb = persist.tile([128, HP, S], bf16, tag="qT")
        kT_sb = persist.tile([128, HP, S], bf16, tag="kT")
        v_sb = persist.tile([128, ST, DH], bf16, tag="v")
        wo_sb = persist.tile([128, HP, DM], bf16, tag="wo")
        bq_sb = persist.tile([128, HP], f32, tag="bq")
        bk_sb = persist.tile([128, HP], f32, tag="bk")

        # ---------------- phase 1: load + QKV projections ----------------
        with tc.tile_pool(name="ph1", bufs=1) as ph1, \
             tc.tile_pool(name="psum1", bufs=4, space="PSUM") as psum1:
            xT_sb = ph1.tile([128, CT, S], bf16, tag="xT")
            wq_sb = ph1.tile([128, CT, DH], bf16, tag="wq")
            wk_sb = ph1.tile([128, CT, DH], bf16, tag="wk")
            wv_sb = ph1.tile([128, CT, DH], bf16, tag="wv")

            nc.sync.dma_start(xT_sb[:], xT.rearrange("(t p) s -> p t s", p=128))
            nc.sync.dma_start(wq_sb[:], wq.rearrange("(t p) d -> p t d", p=128))
            nc.sync.dma_start(wk_sb[:], wk.rearrange("(t p) d -> p t d", p=128))
            nc.sync.dma_start(wv_sb[:], wv.rearrange("(t p) d -> p t d", p=128))
            nc.sync.dma_start(wo_sb[:], wo.rearrange("(t p) e -> p t e", p=128))
            nc.sync.dma_start(bq_sb[:], bq.rearrange("(t p) -> p t", p=128))
            nc.sync.dma_start(bk_sb[:], bk.rearrange("(t p) -> p t", p=128))

            # Q^T and K^T: [d, s] layout. out[dt, sc] += w[ct,dt].T @ xT[ct,sc]
            for (w_sb, t_sb, b_sb) in ((wq_sb, qT_sb, bq_sb),
                                       (wk_sb, kT_sb, bk_sb)):
                for hp in range(HP):
                    pss = [psum1.tile([128, 512], f32, tag="ps1",
                                      name=f"ps1_{sc}")
                           for sc in range(SC)]
                    for ct in range(CT):
                        for sc in range(SC):
                            nc.tensor.matmul(
                                pss[sc][:],
                                w_sb[:, ct, hp * 128:(hp + 1) * 128],
                                xT_sb[:, ct, sc * 512:(sc + 1) * 512],
                                start=(ct == 0), stop=(ct == CT - 1))
                    for sc in range(SC):
                        nc.scalar.activation(
                            t_sb[:, hp, sc * 512:(sc + 1) * 512], pss[sc][:],
                            Ident, bias=b_sb[:, hp:hp + 1])
            # V: [s, d] layout. out[st] += xT[ct, st].T @ wv[ct]
            for st in range(ST):
                psv = psum1.tile([128, 512], f32, tag="ps1")
                for ct in range(CT):
                    nc.tensor.matmul(
                        psv[:, :DH],
                        xT_sb[:, ct, st * 128:(st + 1) * 128],
                        wv_sb[:, ct, :],
                        start=(ct == 0), stop=(ct == CT - 1))
                nc.vector.tensor_copy(v_sb[:, st, :], psv[:, :DH])

        # ---------------- phase 2: attention ----------------
        with tc.tile_pool(name="att", bufs=4) as att, \
             tc.tile_pool(name="pT", bufs=3) as pTp, \
             tc.tile_pool(name="pn", bufs=6) as pnp, \
             tc.tile_pool(name="avn", bufs=6) as avnp, \
             tc.tile_pool(name="small", bufs=12) as small, \
             tc.tile_pool(name="ysb", bufs=4) as ysbp, \
             tc.tile_pool(name="ps_sc", bufs=2, space="PSUM") as ps_sc, \
             tc.tile_pool(name="ps_av", bufs=2, space="PSUM") as ps_av, \
             tc.tile_pool(name="ps_y", bufs=2, space="PSUM") as ps_y:
            for qc in range(SC):
                avn_qc = []
                for hp in range(HP):
                    pT_pair = []
                    for hl in range(2):
                        h = 2 * hp + hl
                        off = hl * 64
                        pT_tile = pTp.tile([128, 4, ST, 128], bf16,
                                           tag="pT")
                        for q4 in range(4):
                            qt = qc * 4 + q4
                            exp_sb = att.tile([128, S], bf16, tag="exp")
                            NH = max(1, S // 1024)
                            HS = S // NH
                            acc = small.tile([128, NH], f32, tag="acc")
                            for half in range(NH):
                                ps_s = ps_sc.tile([128, HS], f32, tag="sc",
                                                  name=f"ps_s{half}")
                                for kc in range(HS // 512):
                                    k0 = half * HS + kc * 512
                                    nc.tensor.matmul(
                                        ps_s[:, kc * 512:(kc + 1) * 512],
                                        qT_sb[off:off + 64, hp,
                                              qt * 128:(qt + 1) * 128],
                                        kT_sb[off:off + 64, hp, k0:k0 + 512],
                                        start=True, stop=True,
                                        tile_position=(off, 0))
                                nc.scalar.activation(
                                    exp_sb[:, half * HS:(half + 1) * HS],
                                    ps_s[:], Exp,
                                    accum_out=acc[:, half:half + 1])
                            if NH == 2:
                                sums = small.tile([128, 1], f32, tag="sums")
                                nc.scalar.activation(sums[:], acc[:, 0:1],
                                                     Ident, bias=acc[:, 1:2])
                            else:
                                sums = acc
                            inv = small.tile([128, 1], f32, tag="inv")
                            nc.vector.reciprocal(inv[:], sums[:, 0:1])
                            pn = pnp.tile([128, S], bf16, tag="pn")
                            nc.vector.tensor_scalar(pn[:], exp_sb[:], inv[:],
                                                    None, mult)
                            # probs out (SWDGE casts bf16 -> fp32)
                            nc.gpsimd.dma_start(
                                probs_p[h, qt * 128:(qt + 1) * 128, :], pn[:])
                            # P^T tiles for AV (contiguous xbar dest)
                            nc.sync.dma_start_transpose(
                                pT_tile[:, q4], pn[:])
                        pT_pair.append(pT_tile)
                    # AV for this head pair, col-packed (M=64 each)
                    psa = ps_av.tile([128, 512], f32, tag="av")
                    for kt in range(ST):
                        for hl in range(2):
                            nc.tensor.matmul(
                                psa[hl * 64:(hl + 1) * 64, :],
                                v_sb[:, kt,
                                     hp * 128 + hl * 64:hp * 128 + hl * 64 + 64],
                                pT_pair[hl][:, :, kt, :],
                                start=(kt == 0), stop=(kt == ST - 1),
                                tile_position=(0, hl * 64),
                                skip_group_check=(hl == 1))
                    avn = avnp.tile([128, 512], bf16, tag="avn")
                    nc.vector.tensor_copy(avn[:], psa[:])
                    avn_qc.append(avn)
                # output projection for this q-chunk
                for q4 in range(4):
                    qt = qc * 4 + q4
                    for ec in range(EC):
                        psy = ps_y.tile([128, 512], f32, tag="y")
                        for hp in range(HP):
                            nc.tensor.matmul(
                                psy[:],
                                avn_qc[hp][:, q4 * 128:(q4 + 1) * 128],
                                wo_sb[:, hp, ec * 512:(ec + 1) * 512],
                                start=(hp == 0), stop=(hp == HP - 1))
                        y_sb = ysbp.tile([128, 512], f32, tag="ysb")
                        nc.vector.tensor_copy(y_sb[:], psy[:])
                        nc.sync.dma_start(
                            y_p[qt * 128:(qt + 1) * 128,
                                ec * 512:(ec + 1) * 512], y_sb[:])
    nc.compile()
    return nc


def make_core_inputs(x, Wq, bq, Wk, bk, Wv, bv, Wo, bo):
    """Per-core input dicts. Core c: batch c//2, head-group c%2."""
    scale = np.float32(1.0 / np.sqrt(D_K))
    xT = [np.ascontiguousarray(x[b].T).astype(BF16) for b in range(B)]
    per_hg = []
    for hg in range(2):
        ds = slice(hg * 512, hg * 512 + 512)
        per_hg.append({
            "wq": np.ascontiguousarray(Wq[:, ds] * scale).astype(BF16),
            "wk": np.ascontiguousarray(Wk[:, ds]).astype(BF16),
            "wv": np.ascontiguousarray(Wv[:, ds]).astype(BF16),
            "wo": np.ascontiguousarray(Wo[ds, :]).astype(BF16),
            "bq": np.ascontiguousarray(bq[ds] * scale).astype(np.float32),
            "bk": np.ascontiguousarray(bk[ds]).astype(np.float32),
        })
    in_maps = []
    for c in range(N_CORES):
        b, hg = c // 2, c % 2
        m = {"xT": xT[b]}
        m.update(per_hg[hg])
        in_maps.append(m)
    return in_maps


def _reference_numpy(x, mask, Wq, bq, Wk, bk, Wv, bv, Wo, bo):
    """Exact fp32 fallback (only used for inputs the device path can't take)."""
    Bn, S, _ = x.shape
    H, dk = NUM_HEADS, D_K

    def split(t):
        return t.reshape(Bn, S, H, dk).transpose(0, 2, 1, 3)

    Q = split(x @ Wq + bq)
    K = split(x @ Wk + bk)
    V = split(x @ Wv + bv)
    scores = np.einsum("bhqd,bhkd->bhqk", Q, K) / np.float32(np.sqrt(dk))
    scores = np.where(mask[:, None, :, :] == 0, np.float32(-1e9), scores)
    scores -= scores.max(axis=-1, keepdims=True)
    e = np.exp(scores)
    probs = e / e.sum(axis=-1, keepdims=True)
    out = np.einsum("bhqk,bhkd->bhqd", probs, V)
    out = out.transpose(0, 2, 1, 3).reshape(Bn, S, H * dk)
    out = out @ Wo + bo
    return out.astype(np.float32), probs.astype(np.float32)


_NC_CACHE = {}


def kernel(x, mask, Wq, bq, Wk, bk, Wv, bv, Wo, bo, _want_results=False):
    x = np.asarray(x)
    mask = np.asarray(mask)
    Wq, bq = np.asarray(Wq), np.asarray(bq)
    Wk, bk = np.asarray(Wk), np.asarray(bk)
    Wv, bv = np.asarray(Wv), np.asarray(bv)
    Wo, bo = np.asarray(Wo), np.asarray(bo)

    if (x.shape != (B, S_FULL, D_MODEL)) or (mask == 0).any():
        return _reference_numpy(x, mask, Wq, bq, Wk, bk, Wv, bv, Wo, bo)

    from concourse.bass_utils import run_bass_kernel_spmd

    if "nc" not in _NC_CACHE:
        _NC_CACHE["nc"] = build_bass()
    nc = _NC_CACHE["nc"]

    in_maps = make_core_inputs(x, Wq, bq, Wk, bk, Wv, bv, Wo, bo)
    res = run_bass_kernel_spmd(nc, in_maps, core_ids=list(range(N_CORES)))

    probs = np.empty((B, NUM_HEADS, S_FULL, S_FULL), np.float32)
    out = np.empty((B, S_FULL, D_MODEL), np.float32)
    host_bias = (bv.astype(np.float32) @ Wo.astype(np.float32)
                 + bo.astype(np.float32))
    for c in range(N_CORES):
        b, hg = c // 2, c % 2
        probs[b, hg * 8:hg * 8 + 8] = res.results[c]["probs_p"]
    for b in range(B):
        out[b] = (res.results[2 * b]["y_p"] + res.results[2 * b + 1]["y_p"]
                  + host_bias)
    if _want_results:
        return (out, probs), res
    return (out, probs)
